# revision 38
# baseline (speedup 1.0000x reference)
"""CapsNet forward kernel for Trainium2, 8-core data-parallel (v3, bf16).

Strategy: batch (512) split across 8 cores (64 each); params replicated.
Routing logits b are a batch-mean -> ReduceScatter of 8x-replicated
per-core partial deltas (1152 floats) per routing round (rounds 0,1;
round 2's b update is dead).

v3 changes vs v2 (HW slope ~650us -> ~520-550us):
  - h1 stored in polyphase/quadrant layout (plane q=py*2+px holds pixels
    (2ry+py, 2rx+px) at img*100+ry*10+rx): conv2's moving-operand AP
    becomes [[100,nb],[10,6],[1,6]] with a ~2.8KB span.  Measured on HW:
    wide-span strided rhs ([[400,nb],[40,6],[2,6]]) streams ~70ns/MM
    slower than contiguous; the quadrant layout recovers full rate
    (~-80us on HW, invisible to the cost-model sim).
  - conv1 emits 2 row-parity matmuls (N=200) per (img, ci) matching the
    quadrant psum layout; single bias+relu per (img, ci) writes all 4
    planes via a [[3200,4],[1,100]] AP.
  - pc_rd split per conv half so pc2 stores are fully contiguous (128
    segments); xrT gathers issued on the Activation hwdge queue, in
    parallel with stores on the SP queue; the last (half1,co1) block
    streams through 4 piecewise store->load pairs (9 r-tiles each) so
    the round-0 s-chain tail unblocks incrementally.
  - xrT merged into one [128, NT*BC] tile: delta's P*xr reduction is 9
    full-PSUM-bank groups (8 r-tiles each) consumed by ONE fused
    multiply+accum DVE op per j (was 18 half-bank ops + extra reduce).
  - scaled_x single xc tile, quarters in s-chain consumption order
    (DVE takes 3 incl. the head, Pool takes the later third quarter).
  - round-1 softmax exp reads the collective result directly with
    scale=ROUTE_SCALE (b0 = 0), keeping the b9 bookkeeping mul off the
    critical path; (half1,co1) pc2 stores are issued per
    (partition-piece x image-group) as each group's bias+relu lands, so
    only the 4-image last-group blocks gate the final xrT gathers.
"""

import numpy as np

import concourse.bass as bass
import concourse.bass_isa as bass_isa
import concourse.mybir as mybir
import concourse.tile as tile
from concourse.ap import AP
from concourse.bass_utils import run_bass_kernel_spmd

F32 = mybir.dt.float32
BF16 = mybir.dt.bfloat16
AL = mybir.AluOpType
AF = mybir.ActivationFunctionType
AX = mybir.AxisListType

import os
_NO_COLLECTIVE = bool(os.environ.get("KERNEL_NO_COLLECTIVE"))
_CONV_ONLY = bool(os.environ.get("KERNEL_CONV_ONLY"))
_ROUTE_ONLY = bool(os.environ.get("KERNEL_ROUTE_ONLY"))

NCORES = 8
B = 512
BC = B // NCORES           # 64 images per core
HB = BC // 2               # 32 images per conv half
MAX_WAITS = 1              # walrus on this path allows 1 sync wait per inst
HL = 160                   # 10 classes x 16 pose
NS = 9216                  # 1152 caps x 8
NT = NS // 128             # 72 K-tiles
HT = NT // 2               # 36 K-tiles per xrT half
GROUPS = [(0, 14), (14, 14), (28, 4)]   # conv2 image groups per half
PATCH_CHUNKS = [(0, 7), (7, 7), (14, 7), (21, 7), (28, 4)]  # conv1 DMA chunks
ROUTE_SCALE = 1.0 / (B * HL)


def _r(t, dims):
    """Raw AP on tile/ap t with explicit [step, count] dims (elements)."""
    return AP(t.tensor, t.offset, dims)


def split_waits(nc, max_waits=MAX_WAITS):
    """This walrus build rejects >max_waits sync waits per instruction; move
    excess waits onto same-engine NoOps inserted immediately before."""
    for f in nc.m.functions:
        for blk in f.blocks:
            out = []
            for ins in blk.instructions:
                si = ins.sync_info
                if si is not None and si.on_wait and len(si.on_wait) > max_waits:
                    waits = list(si.on_wait)
                    k = 0
                    while len(waits) > max_waits:
                        chunk, waits = waits[:max_waits], waits[max_waits:]
                        nop = mybir.InstNoOp(name=f"{ins.name}-ws{k}", ins=[], outs=[])
                        nop.engine = ins.engine
                        nop.sync_info = mybir.SyncInfo(on_wait=chunk, on_update=[])
                        out.append(nop)
                        k += 1
                    ins.sync_info = mybir.SyncInfo(
                        on_wait=waits, on_update=list(si.on_update or []))
                out.append(ins)
            blk.instructions = out


def build_nc(repeat=1):
    nc = bass.Bass(num_devices=NCORES)

    xp = nc.dram_tensor("xp", [81, BC, 560], BF16, kind="ExternalInput")
    w1t = nc.dram_tensor("w1t", [81, 256], BF16, kind="ExternalInput")
    b1 = nc.dram_tensor("b1", [256], F32, kind="ExternalInput")
    pcw4 = nc.dram_tensor("pcw4", [4, 128, 81 * 128], BF16, kind="ExternalInput")
    pcb = nc.dram_tensor("pcb", [256], F32, kind="ExternalInput")
    w2ns = nc.dram_tensor("w2ns", [128, NT * HL], BF16, kind="ExternalInput")
    w2nt = nc.dram_tensor("w2nt", [HL, NS], BF16, kind="ExternalInput")
    eye64 = nc.dram_tensor("eye64", [BC, BC], BF16, kind="ExternalInput")
    vout = nc.dram_tensor("vout", [BC, HL], F32, kind="ExternalOutput")

    # per conv half so [r, b-half] stores from pc2 are fully contiguous
    pc_rd = [nc.dram_tensor(f"pc_rd{h}", [NS, HB], BF16) for h in range(2)]

    with tile.TileContext(nc) as tc:
        with (
            tc.tile_pool(name="pers", bufs=1) as pers,
            tc.tile_pool(name="dram", bufs=1, space="DRAM") as dpool,
        ):
            w1t_sb = pers.tile([81, 256], BF16)
            nc.sync.dma_start(w1t_sb[:], w1t[:])
            b1_sb = pers.tile([128, 2], F32)
            pcb_sb = pers.tile([128, 2], F32)
            eye_sb = pers.tile([BC, BC], BF16)
            zero1 = pers.tile([128, 1], F32)
            nc.gpsimd.memset(zero1[:], 0.0)
            ones128 = pers.tile([128, 1], F32)
            nc.gpsimd.memset(ones128[:], 1.0)
            ones1 = pers.tile([1, 128], F32)
            nc.gpsimd.memset(ones1[:], 1.0)
            b9 = pers.tile([128, 9], F32)
            # big persistent routing tensors (DMAs issued later, mid-conv)
            w2sb = pers.tile([128, NT * HL], BF16)
            xrT = pers.tile([128, NT * BC], BF16)

            for _it in range(repeat):
                _body(nc, tc, dpool, _it, xp, b1, pcw4, pcb, w2ns, w2nt,
                      eye64, vout, pc_rd, w1t_sb, b1_sb, pcb_sb, eye_sb,
                      zero1, ones128, ones1, b9, w2sb, xrT)

    return nc


def _body(nc, tc, dpool, _it, xp, b1, pcw4, pcb, w2ns, w2nt, eye64, vout,
          pc_rd, w1t_sb, b1_sb, pcb_sb, eye_sb, zero1, ones128, ones1, b9,
          w2sb, xrT):
    if True:
        if True:
            with tc.tile_pool(name="wbig", bufs=1) as wbig:
                if _ROUTE_ONLY:
                    w2nt_a = wbig.tile([128, NT * 128], BF16, tag="wslot",
                                       bufs=4, name="w2nt_a")
                    nc.sync.dma_start(
                        w2nt_a[:],
                        AP(w2nt[:].tensor, 0,
                           [[NS, 128], [128, NT], [1, 128]]))
                    w2nt_b = wbig.tile([32, NT * 128], BF16, tag="wslot",
                                       bufs=4, name="w2nt_b")
                    nc.sync.dma_start(
                        w2nt_b[:],
                        AP(w2nt[:].tensor, 128 * NS,
                           [[NS, 32], [128, NT], [1, 128]]))
                    nc.sync.dma_start(w2sb[:], w2ns[:])
                    nc.sync.dma_start(eye_sb[:], eye64[:])
                    nc.gpsimd.memset(xrT[:], 0.01)
                    _route(nc, tc, dpool, _it, vout, eye_sb, zero1, ones128,
                           ones1, b9, w2sb, xrT, w2nt_a, w2nt_b)
                    return
                w2nt_a, w2nt_b = _conv(
                    nc, tc, dpool, _it, xp, b1, pcw4, pcb, w2ns, w2nt,
                    eye64, vout, pc_rd, w1t_sb, b1_sb, pcb_sb, eye_sb,
                    zero1, ones128, ones1, b9, w2sb, xrT, wbig)
                if _CONV_ONLY:
                    with tc.tile_pool(name="rnd", bufs=1) as rnd:
                        v0 = rnd.tile([BC, HL], F32)
                        nc.gpsimd.memset(v0[:], 0.01)
                        nc.sync.dma_start(vout[:], v0[:])
                else:
                    _route(nc, tc, dpool, _it, vout, eye_sb, zero1,
                           ones128, ones1, b9, w2sb, xrT,
                           w2nt_a, w2nt_b)


def _conv(nc, tc, dpool, _it, xp, b1, pcw4, pcb, w2ns, w2nt, eye64, vout,
          pc_rd, w1t_sb, b1_sb, pcb_sb, eye_sb, zero1, ones128, ones1, b9,
          w2sb, xrT, wbig):
    if True:
        if True:
            if True:
                # ---------------- conv phase ----------------
                with (
                    tc.tile_pool(name="convsb", bufs=1) as csb,
                    tc.tile_pool(name="pwp", bufs=2) as pwp,
                    tc.tile_pool(name="pc2p", bufs=2) as pc2p,
                    tc.tile_pool(name="ps1p", bufs=4, space="PSUM") as ps1p,
                    tc.tile_pool(name="ps2p", bufs=2, space="PSUM") as ps2p,
                ):
                    h1_0 = csb.tile([128, HB * 400], BF16)
                    h1_1 = csb.tile([128, HB * 400], BF16)
                    h1 = [h1_0, h1_1]
                    w2c = {}
                    for (co, ci) in [(0, 0), (0, 1), (1, 0), (1, 1)]:
                        t = wbig.tile([128, 81 * 128], BF16, tag="wslot",
                                      bufs=4, name=f"w2c_{co}{ci}")
                        w2c[(co, ci)] = t

                    def load_w2c(co, ci, split=1):
                        base = (co * 2 + ci) * 128 * 81 * 128
                        kk0 = 0
                        for s in range(split):
                            nkk = (81 - kk0) // (split - s)
                            nc.sync.dma_start(
                                w2c[(co, ci)][:, kk0 * 128:(kk0 + nkk) * 128],
                                AP(pcw4[:].tensor, base + kk0 * 128,
                                   [[81 * 128, 128], [1, nkk * 128]]),
                            )
                            kk0 += nkk

                    def patch_dma(half, lo, n):
                        """DMA patch rows for images half*32+lo .. +n."""
                        pw = pwp.tile([81, 8 * 560], BF16, tag="pw")
                        nc.sync.dma_start(
                            _r(pw, [[pw.ap[0][0], 81], [1, n * 560]]),
                            AP(xp[:].tensor, (half * HB + lo) * 560,
                               [[BC * 560, 81], [560, n], [1, 560]]),
                        )
                        return pw

                    def conv1_imgs(pw, lo, n):
                        """conv1 matmuls/acts for the n images in patch pw.

                        h1 is stored in polyphase (quadrant) layout: plane
                        q=py*2+px at offset q*HB*100 holds pixels
                        (2ry+py, 2rx+px) as img*100 + ry*10 + rx. This keeps
                        conv2's moving-operand span small (fast streaming).
                        """
                        pwstep = pw.ap[0][0]
                        for j in range(n):
                            for ci in range(2):
                                ps1 = ps1p.tile([128, 400], F32, tag="ps1")
                                p1s = ps1.ap[0][0]
                                for py in range(2):
                                    rhs = AP(pw.tensor,
                                             pw.offset + j * 560 + py * 28,
                                             [[pwstep, 81], [1, 2],
                                              [56, 10], [2, 10]])
                                    out4 = AP(ps1.tensor,
                                              ps1.offset + py * 200,
                                              [[p1s, 128], [100, 2], [1, 100]])
                                    nc.tensor.matmul(
                                        out4,
                                        w1t_sb[:, ci * 128:(ci + 1) * 128],
                                        rhs,
                                        start=True, stop=True,
                                        skip_group_check=True,
                                    )
                                il = lo + j
                                dst = AP(h1[ci].tensor,
                                         h1[ci].offset + il * 100,
                                         [[h1[ci].ap[0][0], 128],
                                          [HB * 100, 4], [1, 100]])
                                if ci == 0:
                                    nc.scalar.activation(
                                        dst, ps1[:], AF.Relu,
                                        bias=b1_sb[:, 0:1],
                                    )
                                else:
                                    nc.vector.scalar_tensor_tensor(
                                        dst, ps1[:], b1_sb[:, 1:2],
                                        _r(zero1, [[zero1.ap[0][0], 128], [0, 400]]),
                                        AL.add, AL.max,
                                    )

                    def conv2_chain(half, co, pc2, group, ci):
                        """Half of a K=256 conv2 PSUM chain (one ci block)."""
                        g0, nb = group
                        key = (half, co, g0)
                        if ci == 0:
                            ps2 = ps2p.tile([128, 504], F32, tag="ps2")
                            _ps2_open[key] = ps2
                        else:
                            ps2 = _ps2_open.pop(key)
                        pstep = ps2.ap[0][0]
                        wt = w2c[(co, ci)]
                        hp = h1[ci].ap[0][0]
                        for kk in range(81):
                            ky, kx = divmod(kk, 9)
                            q = (ky % 2) * 2 + (kx % 2)
                            rhs = AP(h1[ci].tensor,
                                     h1[ci].offset + q * HB * 100 + g0 * 100
                                     + (ky // 2) * 10 + (kx // 2),
                                     [[hp, 128], [100, nb], [10, 6], [1, 6]])
                            out4 = _r(ps2, [[pstep, 128], [36, nb],
                                            [6, 6], [1, 6]])
                            nc.tensor.matmul(
                                out4,
                                wt[:, kk * 128:(kk + 1) * 128],
                                rhs,
                                start=(ci == 0 and kk == 0),
                                stop=(ci == 1 and kk == 80),
                            )
                        if ci == 1:
                            # bias+relu, pix-major into pc2 [p, pix*32+b]
                            nc.scalar.activation(
                                AP(pc2.tensor, pc2.offset + g0,
                                   [[pc2.ap[0][0], 128], [1, nb], [HB, 36]]),
                                _r(ps2, [[pstep, 128], [36, nb], [1, 36]]),
                                AF.Relu,
                                bias=pcb_sb[:, co:co + 1],
                            )

                    _ps2_open = {}

                    def conv2_group(half, co, pc2, g0, nb):
                        conv2_chain(half, co, pc2, (g0, nb), ci=0)
                        conv2_chain(half, co, pc2, (g0, nb), ci=1)

                    def store_pc2(half, co, pc2):
                        """Store a (half, co) pc2 block into pc_rd[half]:
                        fully contiguous on both sides (128 segments)."""
                        nc.sync.dma_start(
                            AP(pc_rd[half][:].tensor, co * 128 * 36 * HB,
                               [[36 * HB, 128], [1, 36 * HB]]),
                            AP(pc2.tensor, pc2.offset,
                               [[pc2.ap[0][0], 128], [1, 36 * HB]]),
                        )

                    def load_xrT(tbase, half):
                        """Gather xrT columns for 36 r-tiles at tile tbase
                        from pc_rd[half], on the ACT hwdge queue (parallel
                        to the SP store queue)."""
                        nc.scalar.dma_start(
                            AP(xrT.tensor,
                               xrT.offset + tbase * BC + half * HB,
                               [[xrT.ap[0][0], 128], [BC, HT], [1, HB]]),
                            AP(pc_rd[half][:].tensor, tbase * 128 * HB,
                               [[HB, 128], [128 * HB, HT], [1, HB]]),
                        )

                    def conv_half(half, pws, mid=None, on_co1_group=None):
                        # conv1 images 0..13 (chunks 0,1)
                        for (pw, lo, n) in pws[:2]:
                            conv1_imgs(pw, lo, n)
                        pc2 = pc2p.tile([128, HB * 36], BF16, tag="pc2")
                        # g0 (images 0..13) as soon as w2c00 lands;
                        # conv1 images 14..27 interleave between its chains
                        conv2_chain(half, 0, pc2, GROUPS[0], ci=0)
                        for (pw, lo, n) in pws[2:4]:
                            conv1_imgs(pw, lo, n)
                        conv2_chain(half, 0, pc2, GROUPS[0], ci=1)
                        conv2_group(half, 0, pc2, *GROUPS[1])
                        for (pw, lo, n) in pws[4:]:
                            conv1_imgs(pw, lo, n)
                        conv2_group(half, 0, pc2, *GROUPS[2])
                        store_pc2(half, 0, pc2)
                        # co-0 rows of this half are now in flight: pull the
                        # matching xrT_a columns while co-1 computes
                        load_xrT(0, half)
                        if mid is not None:
                            mid()
                        pc2 = pc2p.tile([128, HB * 36], BF16, tag="pc2")
                        p2s = pc2.ap[0][0]
                        for (g0, nb) in GROUPS:
                            conv2_group(half, 1, pc2, g0, nb)
                            if half == 1:
                                # store each (partition-piece x image-group)
                                # block as soon as this group's ACT lands, so
                                # only the tiny last-group blocks gate the
                                # xrT gathers at the conv/routing boundary
                                for q in range(4):
                                    nc.sync.dma_start(
                                        AP(pc_rd[1][:].tensor,
                                           (128 + 32 * q) * 36 * HB + g0,
                                           [[36 * HB, 32], [HB, 36],
                                            [1, nb]]),
                                        AP(pc2.tensor,
                                           pc2.offset + 32 * q * p2s + g0,
                                           [[p2s, 32], [HB, 36], [1, nb]]),
                                    )
                        if half == 0:
                            store_pc2(half, 1, pc2)
                            load_xrT(HT, half)
                        else:
                            xbs = xrT.ap[0][0]
                            for q in range(4):
                                nc.scalar.dma_start(
                                    AP(xrT.tensor,
                                       xrT.offset + (HT + 9 * q) * BC + HB,
                                       [[xbs, 128], [BC, 9], [1, HB]]),
                                    AP(pc_rd[1][:].tensor,
                                       (4608 + 1152 * q) * HB,
                                       [[HB, 128], [128 * HB, 9], [1, HB]]),
                                )
                        return pc2

                    # ---- half 0 (patch DMAs interleaved with w2c loads) ----
                    pws0 = []
                    for (lo, n) in PATCH_CHUNKS[:2]:
                        pws0.append((patch_dma(0, lo, n), lo, n))
                    nc.sync.dma_start(b1_sb[:], _r(b1[:], [[1, 128], [128, 2]]))
                    nc.sync.dma_start(pcb_sb[:], _r(pcb[:], [[1, 128], [128, 2]]))
                    load_w2c(0, 0)
                    for (lo, n) in PATCH_CHUNKS[2:4]:
                        pws0.append((patch_dma(0, lo, n), lo, n))
                    load_w2c(0, 1)
                    pws0.append((patch_dma(0, *PATCH_CHUNKS[4]), *PATCH_CHUNKS[4]))

                    def mid0():
                        # co1 weights + routing prefetches, issued after the
                        # half-0 co0 section in program order
                        load_w2c(1, 0)
                        load_w2c(1, 1)
                        nc.sync.dma_start(w2sb[:], w2ns[:])
                        nc.sync.dma_start(eye_sb[:], eye64[:])

                    conv_half(0, pws0, mid=mid0)
                    # ---- half 1 (first two patch chunks pre-issued) ----
                    pws1 = [(patch_dma(1, lo, n), lo, n)
                            for (lo, n) in PATCH_CHUNKS]
                    w2nt_ab = []

                    def mid1():
                        # w2nt into the conv-weight slots freed by co 0
                        w2nt_a = wbig.tile([128, NT * 128], BF16, tag="wslot",
                                           bufs=4, name="w2nt_a")
                        nc.sync.dma_start(
                            w2nt_a[:],
                            AP(w2nt[:].tensor, 0,
                               [[NS, 128], [128, NT], [1, 128]]))
                        w2nt_b = wbig.tile([32, NT * 128], BF16, tag="wslot",
                                           bufs=4, name="w2nt_b")
                        nc.sync.dma_start(
                            w2nt_b[:],
                            AP(w2nt[:].tensor, 128 * NS,
                               [[NS, 32], [128, NT], [1, 128]]))
                        w2nt_ab.extend([w2nt_a, w2nt_b])

                    conv_half(1, pws1, mid=mid1)
                return w2nt_ab[0], w2nt_ab[1]


def _route(nc, tc, dpool, _it, vout, eye_sb, zero1, ones128, ones1, b9,
           w2sb, xrT, w2nt_a, w2nt_b):
    if True:
        if True:
            if True:
                # ---------------- routing phase ----------------
                with (
                    tc.tile_pool(name="rnd", bufs=2) as rnd,
                    tc.tile_pool(name="sps", bufs=1, space="PSUM") as sps,
                    tc.tile_pool(name="gps", bufs=3, space="PSUM") as gps,
                    tc.tile_pool(name="vps", bufs=1, space="PSUM") as vps,
                    tc.tile_pool(name="zps", bufs=1, space="PSUM") as zps,
                ):
                    def s_matmul(xst_of):
                        s_ps = sps.tile([BC, HL], F32, tag="s_ps")
                        for t in range(NT):
                            nc.tensor.matmul(
                                s_ps[:],
                                xst_of(t),
                                w2sb[:, t * HL:(t + 1) * HL],
                                start=(t == 0), stop=(t == NT - 1),
                            )
                        return s_ps

                    def xr_slice(t):
                        return xrT[:, t * BC:(t + 1) * BC]

                    def squash(s_ps, out_dtype, scale):
                        """v = squash(s_ps*scale) over the class dim.
                        scale may be a float or a per-partition [BC,1] AP
                        (the runtime 1/Z of the softmax, applied here so
                        the s-chain never waits for the Z reduction)."""
                        sq = rnd.tile([BC, HL], F32, tag="sq")
                        nc.scalar.activation(sq[:], s_ps[:], AF.Square,
                                             scale=scale)
                        n2 = rnd.tile([BC, 16], F32, tag="n2")
                        nc.vector.tensor_reduce(
                            n2[:].rearrange("a b -> a b ()"),
                            _r(sq, [[sq.ap[0][0], BC], [1, 16], [16, 10]]),
                            AX.X, AL.add,
                        )
                        rt = rnd.tile([BC, 16], F32, tag="rt")
                        nc.scalar.sqrt(rt[:], n2[:])
                        n2p1 = rnd.tile([BC, 16], F32, tag="n2p1")
                        nc.vector.tensor_scalar_add(n2p1[:], n2[:], 1.0)
                        rcp = rnd.tile([BC, 16], F32, tag="rcp")
                        nc.vector.reciprocal(rcp[:], n2p1[:])
                        f = rnd.tile([BC, 16], F32, tag="f")
                        nc.vector.tensor_tensor(f[:], rt[:], rcp[:], AL.mult)
                        v_sb = rnd.tile([BC, HL], out_dtype, tag="v_sb")
                        nc.vector.scalar_tensor_tensor(
                            _r(v_sb, [[v_sb.ap[0][0], BC], [16, 10], [1, 16]]),
                            _r(s_ps, [[s_ps.ap[0][0], BC], [16, 10], [1, 16]]),
                            scale,
                            _r(f, [[f.ap[0][0], BC], [0, 10], [1, 16]]),
                            AL.mult, AL.mult,
                        )
                        return v_sb

                    def delta_update(v_bf, rnd_idx):
                        """b9 (+)= ROUTE_SCALE * allreduce(sum_b xrT*P)."""
                        vt_ps = vps.tile([128, BC], BF16, tag="vt_ps")
                        nc.tensor.transpose(vt_ps[:], v_bf[:, 0:128], eye_sb[:])
                        vt_a = rnd.tile([128, BC], BF16, tag="vt_a")
                        nc.scalar.copy(vt_a[:], vt_ps[:])
                        vtb_ps = vps.tile([32, BC], BF16, tag="vtb_ps")
                        nc.tensor.transpose(vtb_ps[:], v_bf[:, 128:160], eye_sb[:])
                        vt_b = rnd.tile([32, BC], BF16, tag="vt_b")
                        nc.scalar.copy(vt_b[:], vtb_ps[:])
                        delta9 = rnd.tile([128, 9], F32, tag="delta9")
                        # Group P tiles by j = t%9: 8 r-tiles fill one PSUM
                        # bank; one fused multiply+sum per j yields
                        # delta9[:,j] directly.
                        for j in range(9):
                            pph = gps.tile([128, 8 * BC], F32, tag="pph")
                            for si in range(8):
                                t = si * 9 + j
                                nc.tensor.matmul(
                                    pph[:, si * BC:(si + 1) * BC],
                                    w2nt_a[:, t * 128:(t + 1) * 128],
                                    vt_a[:],
                                    start=True, stop=False,
                                )
                                nc.tensor.matmul(
                                    pph[:, si * BC:(si + 1) * BC],
                                    w2nt_b[:, t * 128:(t + 1) * 128],
                                    vt_b[:],
                                    start=False, stop=True,
                                )
                            prodh = rnd.tile([128, 8 * BC], BF16, tag="prodh")
                            in1 = AP(xrT.tensor, xrT.offset + j * BC,
                                     [[xrT.ap[0][0], 128], [9 * BC, 8],
                                      [1, BC]])
                            in0 = _r(pph, [[pph.ap[0][0], 128], [BC, 8],
                                           [1, BC]])
                            out3 = _r(prodh, [[prodh.ap[0][0], 128],
                                              [BC, 8], [1, BC]])
                            nc.vector.scalar_tensor_tensor(
                                out3, in0, 1.0, in1, AL.mult, AL.mult,
                                accum_out=delta9[:, j:j + 1],
                            )
                        # exchange: ReduceScatter over an 8x-replicated input
                        # -> every core receives the full summed delta
                        crep = dpool.tile([NCORES, 128 * 9], F32,
                                          name=f"crep{rnd_idx}_{_it}")
                        cd = dpool.tile([128 * 9], F32,
                                        name=f"cd{rnd_idx}_{_it}")
                        nc.sync.dma_start(
                            AP(crep[:].tensor, 0,
                               [[9, 128], [128 * 9, NCORES], [1, 9]]),
                            _r(delta9, [[delta9.ap[0][0], 128],
                                        [0, NCORES], [1, 9]]),
                        )
                        if _NO_COLLECTIVE:
                            # timing-only variant: skip the cross-core sync
                            nc.sync.dma_start(
                                AP(cd.tensor, cd.offset, [[1, 128 * 9]]),
                                AP(crep[:].tensor, 0, [[1, 128 * 9]]),
                            )
                        else:
                            nc.gpsimd.collective_compute(
                                "ReduceScatter", AL.add,
                                replica_groups=[list(range(NCORES))],
                                ins=[crep.opt()], outs=[cd.opt()],
                            )
                        dsum = rnd.tile([128, 9], F32, tag="dsum")
                        nc.sync.dma_start(
                            dsum[:],
                            AP(cd.tensor, cd.offset, [[9, 128], [1, 9]]),
                        )
                        if rnd_idx == 0:
                            nc.scalar.mul(b9[:], dsum[:], ROUTE_SCALE)
                        else:
                            nc.vector.scalar_tensor_tensor(
                                b9[:], dsum[:], ROUTE_SCALE, b9[:],
                                AL.mult, AL.add)
                        return dsum

                    def softmax_ce9b(bsrc, bscale):
                        """ce9b[p,j] = softmax(bscale*bsrc)[n=j*128+p].
                        Round 1 passes dsum directly (b0 = 0, so
                        b1 = ROUTE_SCALE*dsum) which skips the serial b9
                        bookkeeping op on the critical path."""
                        e9 = rnd.tile([128, 9], F32, tag="e9")
                        rs9 = rnd.tile([128, 1], F32, tag="rs9")
                        nc.scalar.activation(e9[:], bsrc[:], AF.Exp,
                                             scale=bscale,
                                             accum_out=rs9[:])
                        z_ps = zps.tile([1, 1], F32, tag="z_ps")
                        nc.tensor.matmul(z_ps[:], ones128[:], rs9[:],
                                         start=True, stop=True)
                        z_sb = rnd.tile([1, 1], F32, tag="z_sb")
                        nc.scalar.copy(z_sb[:], z_ps[:])
                        zb_ps = zps.tile([128, 1], F32, tag="zb_ps")
                        nc.tensor.matmul(zb_ps[:], ones1[:], z_sb[:],
                                         start=True, stop=True)
                        rz = rnd.tile([128, 1], F32, tag="rz")
                        nc.vector.reciprocal(rz[:], zb_ps[:])
                        ce9b = rnd.tile([128, 9], BF16, tag="ce9b")
                        nc.vector.tensor_scalar_mul(ce9b[:], e9[:], rz[:])
                        return ce9b

                    def scaled_x(ce9b):
                        """xc[p,(s,j,b)] = xrT * ce9b[p,j], bf16 copy.
                        Quarters in s-chain consumption order; DVE (2x
                        Pool's rate) takes 3 incl. the chain head, Pool
                        takes the third quarter which is needed later."""
                        xc = rnd.tile([128, NT * BC], BF16, tag="xc")
                        for eng, q in ((nc.vector, 0), (nc.vector, 1),
                                       (nc.gpsimd, 2), (nc.vector, 3)):
                            off = q * 2 * 9 * BC
                            eng.tensor_tensor(
                                AP(xc.tensor, xc.offset + off,
                                   [[xc.ap[0][0], 128], [9 * BC, 2],
                                    [BC, 9], [1, BC]]),
                                AP(xrT.tensor, xrT.offset + off,
                                   [[xrT.ap[0][0], 128], [9 * BC, 2],
                                    [BC, 9], [1, BC]]),
                                _r(ce9b, [[ce9b.ap[0][0], 128], [0, 2],
                                          [1, 9], [0, BC]]),
                                AL.mult,
                            )
                        return xc

                    # ---- round 0 (c uniform) ----
                    s_ps = s_matmul(xr_slice)
                    v_bf = squash(s_ps, BF16, 1.0 / 1152.0)
                    dsum0 = delta_update(v_bf, 0)
                    # ---- round 1 ----
                    ce9b = softmax_ce9b(dsum0, ROUTE_SCALE)
                    xc = scaled_x(ce9b)
                    s_ps = s_matmul(lambda t: xc[:, t * BC:(t + 1) * BC])
                    v_bf = squash(s_ps, BF16, 1.0)
                    delta_update(v_bf, 1)
                    # ---- round 2 (b update dead) ----
                    ce9b = softmax_ce9b(b9, 1.0)
                    xc = scaled_x(ce9b)
                    s_ps = s_matmul(lambda t: xc[:, t * BC:(t + 1) * BC])
                    v_sb = squash(s_ps, F32, 1.0)
                    nc.sync.dma_start(vout[:], v_sb[:])


_NC_CACHE = {}


def _get_nc(repeat=1):
    if repeat not in _NC_CACHE:
        nc = build_nc(repeat)
        split_waits(nc)
        _NC_CACHE[repeat] = nc
    return _NC_CACHE[repeat]


def prepare_inputs(x, conv1_w, conv1_b, pc_w, pc_b, W):
    bf = mybir.dt.np(BF16)
    x = np.asarray(x, np.float32)
    xs = np.zeros((B, 800), np.float32)
    xs[:, :784] = x.reshape(B, 784)
    # host-side patch expansion: xp[kk, b, e] = xs[b, (kk//9)*28 + kk%9 + e]
    kidx = (np.arange(9)[:, None] * 28 + np.arange(9)[None, :]).reshape(81)
    xp = np.stack([xs[:, k:k + 560] for k in kidx], 0).astype(bf)  # [81, B, 560]
    w1t = np.ascontiguousarray(
        np.asarray(conv1_w, np.float32).reshape(256, 81).T).astype(bf)
    b1 = np.ascontiguousarray(np.asarray(conv1_b, np.float32))
    pcwt = np.asarray(pc_w, np.float32).reshape(256, 256, 81).transpose(2, 1, 0)
    # pcw4[co*2+ci][p, kk*128+co_p] = pcwt[kk, ci*128+p, co*128+co_p]
    pcw4 = np.stack([
        np.ascontiguousarray(
            pcwt[:, ci * 128:(ci + 1) * 128, co * 128:(co + 1) * 128]
            .transpose(1, 0, 2).reshape(128, 81 * 128))
        for (co, ci) in [(0, 0), (0, 1), (1, 0), (1, 1)]
    ], 0).astype(bf)
    pcb = np.ascontiguousarray(np.asarray(pc_b, np.float32).reshape(256))
    w2n = np.ascontiguousarray(
        np.asarray(W, np.float32).transpose(3, 0, 1, 2).reshape(NS, HL))
    # w2ns[p, t*HL+hl] = w2n[t*128+p, hl]
    w2ns = np.ascontiguousarray(
        w2n.reshape(NT, 128, HL).transpose(1, 0, 2).reshape(128, NT * HL)
    ).astype(bf)
    w2nt = np.ascontiguousarray(w2n.T).astype(bf)
    eye64 = np.eye(BC, dtype=np.float32).astype(bf)
    in_maps = []
    for c in range(NCORES):
        in_maps.append({
            "xp": np.ascontiguousarray(xp[:, c * BC:(c + 1) * BC, :]),
            "w1t": w1t, "b1": b1, "pcw4": pcw4, "pcb": pcb, "w2ns": w2ns,
            "w2nt": w2nt, "eye64": eye64,
        })
    return in_maps


def finalize_output(results):
    v = np.concatenate([np.asarray(results[c]["vout"]) for c in range(NCORES)], 0)
    return v.reshape(B, 1, 1, 10, 16).astype(np.float32)


def kernel(x, conv1_w, conv1_b, pc_w, pc_b, W, _trace=False, _trace_kwargs=None):
    nc = _get_nc()
    in_maps = prepare_inputs(x, conv1_w, conv1_b, pc_w, pc_b, W)
    res = run_bass_kernel_spmd(
        nc, in_maps, list(range(NCORES)),
        trace=_trace, **(_trace_kwargs or {}),
    )
    out = finalize_output(res.results)
    if _trace:
        return out, res
    return out

